# revision 1
# baseline (speedup 1.0000x reference)
"""Trainium2 Bass kernel for nn_KNNDist: mean-5NN-distance outlier loss.

Strategy (pure data parallel, one batch per NeuronCore, 8 cores):
  For each batch b the device computes value[i] = mean of the 5 smallest
  pairwise squared distances from point i to all other points (excluding
  the self-distance), via a single augmented matmul that produces
  negdist[i,j] = 2*pc_i.pc_j - xx_i - xx_j = -dist[i,j] directly in PSUM,
  followed by the DVE top-8 instruction (InstMax) per 512-wide chunk and a
  hierarchical top-8 merge. The tiny final reduction (mean/std/threshold/
  mask/weighting over 4096 values per batch) is done on host in float32
  with the exact reference semantics.

Augmented matmul (contraction K=5):
  lhsT rows: [2x_i, 2y_i, 2z_i, xx_i, -1]
  rhs  rows: [ x_j,  y_j,  z_j,  -1, xx_j]
  => out[i,j] = 2*pc_i.pc_j - xx_i - xx_j  (= -dist[i,j])
"""

import sys
import numpy as np

if "/opt/trn_rl_repo" not in sys.path:
    sys.path.insert(0, "/opt/trn_rl_repo")

import concourse.bass as bass
import concourse.mybir as mybir
import concourse.tile as tile
from concourse import bacc, bass_utils

B = 8          # batches == cores
N = 4096       # points per batch
D = 3          # coordinate dims
K = 5          # augmented contraction dim (fp32 modes)
P = 128        # rows per tile (partition dim)
NT = N // P    # 32 row tiles
CH = 512       # matmul moving-dim chunk (one PSUM bank)
NCH = N // CH  # 8 chunks
KNN = 5
ALPHA = np.float32(1.05)

# mode -> (matmul dtype, contraction dim)
MODES = {
    "float32": ("float32", K),
    "float32r": ("float32r", K),
    "bf16_split": ("bfloat16", 3 * K + 1),  # padded to 16: odd-K bf16 FWL wedged the PE
    "hybrid": ("bfloat16", 3 * K + 1),      # bf16_split matmul + DVE/ACT split scan
}
DEFAULT_MODE = "bf16_split"

_PROGRAM_CACHE = {}


def build_program(mode=DEFAULT_MODE):
    """Build the per-core Bass program (identical on all 8 cores)."""
    dt_name, KK = MODES[mode]
    mm_dtype = getattr(mybir.dt, dt_name)
    f32 = mybir.dt.float32
    nc = bacc.Bacc("TRN2", target_bir_lowering=False, debug=False)
    L = nc.dram_tensor("L", [KK, N], mm_dtype, kind="ExternalInput")
    R = nc.dram_tensor("Rm", [KK, N], mm_dtype, kind="ExternalInput")
    val = nc.dram_tensor("val", [P, NT], f32, kind="ExternalOutput")

    # 4 PSUM banks per scan tile: one DVE max covers 4 matmul chunks,
    # amortizing the ~180ns per-op DVE init/drain overhead
    BPT = 4              # banks (512-col chunks) per psum tile
    NPT = NCH // BPT     # 2 psum tiles per row-tile
    with tile.TileContext(nc) as tc:
        with (
            tc.tile_pool(name="const", bufs=1) as cpool,
            tc.tile_pool(
                name="psum",
                bufs=1 if mode == "hybrid" else 2,
                space=bass.MemorySpace.PSUM,
            ) as psum,
            tc.tile_pool(name="work", bufs=3) as wpool,
        ):
            Ls = cpool.tile([KK, N], mm_dtype, tag="Ls")
            Rs = cpool.tile([KK, N], mm_dtype, tag="Rs")
            vals = cpool.tile([P, NT], f32, tag="vals")
            nc.sync.dma_start(Ls[:], L[:])
            nc.sync.dma_start(Rs[:], R[:])

            bf16 = mybir.dt.bfloat16
            for i in range(NT):
                if mode == "hybrid":
                    # Half the chunks: DVE max8 straight off f32 PSUM.
                    # Other half: ACT converts PSUM->bf16 SBUF, DVE max8
                    # runs in 2x mode on the 2-byte packed data.
                    cand = wpool.tile([P, 16], bf16, tag="cand")
                    psA = psum.tile([P, BPT * CH], f32, tag="psA")
                    for q in range(BPT):
                        nc.tensor.matmul(
                            psA[:, q * CH : (q + 1) * CH],
                            Ls[:, i * P : (i + 1) * P],
                            Rs[:, q * CH : (q + 1) * CH],
                            start=True,
                            stop=True,
                        )
                    nc.vector.max(cand[:, 0:8], psA[:])
                    psB = psum.tile([P, BPT * CH], f32, tag="psB")
                    for q in range(BPT):
                        j = BPT + q
                        nc.tensor.matmul(
                            psB[:, q * CH : (q + 1) * CH],
                            Ls[:, i * P : (i + 1) * P],
                            Rs[:, j * CH : (j + 1) * CH],
                            start=True,
                            stop=True,
                        )
                    sb = wpool.tile([P, BPT * CH], bf16, tag="sb")
                    nc.scalar.activation(
                        sb[:], psB[:], mybir.ActivationFunctionType.Copy
                    )
                    nc.vector.max(cand[:, 8:16], sb[:])
                    top8 = wpool.tile([P, 8], bf16, tag="top8")
                    nc.vector.max(top8[:], cand[:])
                else:
                    cand = wpool.tile([P, NPT * 8], f32, tag="cand")
                    for t in range(NPT):
                        ps = psum.tile([P, BPT * CH], f32, tag="ps")
                        for q in range(BPT):
                            j = t * BPT + q
                            nc.tensor.matmul(
                                ps[:, q * CH : (q + 1) * CH],
                                Ls[:, i * P : (i + 1) * P],
                                Rs[:, j * CH : (j + 1) * CH],
                                start=True,
                                stop=True,
                            )
                        # top-8 largest of -dist == 8 smallest distances
                        nc.vector.max(cand[:, t * 8 : (t + 1) * 8], ps[:])
                    top8 = wpool.tile([P, 8], f32, tag="top8")
                    nc.vector.max(top8[:], cand[:])
                # value = mean(dist of 5 NN) = -(1/5) * sum(top8[:, 1:6])
                scr = wpool.tile([P, KNN], f32, tag="scr")
                nc.scalar.activation(
                    scr[:],
                    top8[:, 1 : 1 + KNN],
                    mybir.ActivationFunctionType.Copy,
                    scale=-1.0 / KNN,
                    accum_out=vals[:, i : i + 1],
                )
            nc.sync.dma_start(val[:], vals[:])
    nc.compile()
    return nc


def get_program(mode=DEFAULT_MODE):
    if mode not in _PROGRAM_CACHE:
        _PROGRAM_CACHE[mode] = build_program(mode)
    return _PROGRAM_CACHE[mode]


def pack_inputs(pc_b, mode=DEFAULT_MODE):
    """Build the [K, N] lhsT / rhs payloads for one batch."""
    p = np.asarray(pc_b, dtype=np.float32)
    xx = np.sum(p * p, axis=1, dtype=np.float32)
    ones = np.ones(N, np.float32)
    Lb = np.ascontiguousarray(
        np.stack([2.0 * p[:, 0], 2.0 * p[:, 1], 2.0 * p[:, 2], xx, -ones])
    ).astype(np.float32)
    Rb = np.ascontiguousarray(
        np.stack([p[:, 0], p[:, 1], p[:, 2], -ones, xx])
    ).astype(np.float32)
    if mode in ("bf16_split", "hybrid"):
        import ml_dtypes

        bf16 = ml_dtypes.bfloat16
        Lh = Lb.astype(bf16)
        Ll = (Lb - Lh.astype(np.float32)).astype(bf16)
        Rh = Rb.astype(bf16)
        Rl = (Rb - Rh.astype(np.float32)).astype(bf16)
        # sum_k L[k] * R[k] = Lh.Rh + Lh.Rl + Ll.Rh  (~fp32 product),
        # plus one zero row padding K to 16
        zero = np.zeros((1, N), bf16)
        Lb = np.ascontiguousarray(np.concatenate([Lh, Lh, Ll, zero], axis=0))
        Rb = np.ascontiguousarray(np.concatenate([Rh, Rl, Rh, zero], axis=0))
    return Lb, Rb


def make_in_maps(pc, mode=DEFAULT_MODE):
    maps = []
    for b in range(B):
        Lb, Rb = pack_inputs(pc[b], mode)
        maps.append({"L": Lb, "Rm": Rb})
    return maps


def finish_on_host(val_tiles, weights):
    """Reference-exact epilogue: threshold stats + weighted mean, in f32."""
    losses = np.zeros(B, np.float32)
    w = np.asarray(weights, dtype=np.float32)
    for b in range(B):
        # val[p, t] holds point index t*128 + p
        v = np.ascontiguousarray(val_tiles[b].T).reshape(-1).astype(np.float32)
        mean = np.mean(v, dtype=np.float32)
        var = np.sum((v - mean) ** 2, dtype=np.float32) / np.float32(N - 1)
        std = np.sqrt(var)
        thr = mean + ALPHA * std
        mask = (v > thr).astype(np.float32)
        losses[b] = np.mean(v * mask, dtype=np.float32) * w[b]
    return np.array(np.mean(losses, dtype=np.float32), dtype=np.float32)


def run_device(pc, mode=DEFAULT_MODE, **spmd_kwargs):
    nc = get_program(mode)
    in_maps = make_in_maps(np.asarray(pc, dtype=np.float32), mode)
    res = bass_utils.run_bass_kernel_spmd(
        nc, in_maps, core_ids=list(range(B)), **spmd_kwargs
    )
    vals = [res.results[b]["val"] for b in range(B)]
    return vals, res


def kernel(pc, weights):
    vals, _ = run_device(pc)
    return finish_on_host(vals, weights)



# revision 2
# speedup vs baseline: 2.7621x; 2.7621x over previous
"""Trainium2 Bass kernel for nn_KNNDist: mean-5NN-distance outlier loss.

Strategy (pure data parallel, one batch per NeuronCore, 8 cores):
  Host sorts each batch's 4096 points along one coordinate axis. For each
  row tile of 128 consecutive sorted points, the device computes negdist
  only against a window of W sorted columns centered on the tile (128-col
  aligned), via the augmented matmul that produces
  negdist[i,j] = 2*pc_i.pc_j - xx_i - xx_j = -dist[i,j] directly in PSUM,
  then one DVE top-8 (InstMax) per tile over the whole window.

  Host epilogue: for each point, if the device 5NN distance d5 is smaller
  than the squared x-distance to the window edge, the windowed top-8 is
  provably the true top-8 (any point outside the window is farther than
  the window edge in x alone). The few points failing the check (~5-10%)
  are recomputed exactly in numpy. Then the reference-exact epilogue
  (mean/std/threshold/mask/weighted mean) in float32/64.

Augmented matmul (contraction K=5, bf16 hi/lo split to K=16):
  lhsT rows: [2x_i, 2y_i, 2z_i, xx_i, -1]
  rhs  rows: [ x_j,  y_j,  z_j,  -1, xx_j]
  => out[i,j] = 2*pc_i.pc_j - xx_i - xx_j  (= -dist[i,j])
"""

import os
import sys
import numpy as np

if "/opt/trn_rl_repo" not in sys.path:
    sys.path.insert(0, "/opt/trn_rl_repo")

import concourse.bass as bass
import concourse.mybir as mybir
import concourse.tile as tile
from concourse import bacc, bass_utils

B = 8          # batches == cores
N = 4096       # points per batch
D = 3          # coordinate dims
KK = 16        # bf16-split contraction rows (15 used + 1 zero pad)
P = 128        # rows per tile (partition dim)
NT = N // P    # 32 row tiles
KNN = 5
ALPHA = np.float32(1.05)
AXIS = 1       # host sort axis

W = int(os.environ.get("BASS_W", "1152"))   # window columns per tile
assert W % 128 == 0

_PROGRAM_CACHE = {}


def window_lo(t, w=W):
    """Start column (128-aligned) of tile t's window."""
    u = w // 128
    lo_u = max(0, min(NT - u, t - (u - 1) // 2))
    return lo_u * P


def build_program(w=W):
    """Per-core Bass program (identical on all 8 cores)."""
    bf16 = mybir.dt.bfloat16
    f32 = mybir.dt.float32
    nbanks = (w + 511) // 512
    nc = bacc.Bacc("TRN2", target_bir_lowering=False, debug=False)
    L = nc.dram_tensor("L", [KK, N], bf16, kind="ExternalInput")
    R = nc.dram_tensor("Rm", [KK, N], bf16, kind="ExternalInput")
    val = nc.dram_tensor("val", [P, NT * 8], f32, kind="ExternalOutput")

    with tile.TileContext(nc) as tc:
        with (
            tc.tile_pool(name="const", bufs=1) as cpool,
            tc.tile_pool(name="psum", bufs=2, space=bass.MemorySpace.PSUM) as psum,
        ):
            Ls = cpool.tile([KK, N], bf16, tag="Ls")
            Rs = cpool.tile([KK, N], bf16, tag="Rs")
            vals = cpool.tile([P, NT * 8], f32, tag="vals")
            nc.sync.dma_start(Ls[:], L[:])
            nc.sync.dma_start(Rs[:], R[:])

            for t in range(NT):
                lo = window_lo(t, w)
                ps = psum.tile([P, nbanks * 512], f32, tag="ps")
                for j in range(nbanks):
                    c0 = j * 512
                    cw = min(512, w - c0)
                    nc.tensor.matmul(
                        ps[:, c0 : c0 + cw],
                        Ls[:, t * P : (t + 1) * P],
                        Rs[:, lo + c0 : lo + c0 + cw],
                        start=True,
                        stop=True,
                    )
                # top-8 largest negdist == 8 smallest distances (incl. self)
                nc.vector.max(vals[:, t * 8 : t * 8 + 8], ps[:, 0:w])
            nc.sync.dma_start(val[:], vals[:])
    nc.compile()
    return nc


def get_program(w=W):
    if w not in _PROGRAM_CACHE:
        _PROGRAM_CACHE[w] = build_program(w)
    return _PROGRAM_CACHE[w]


def prep_batch(pc_b):
    """Sort one batch by AXIS and build the bf16-split [KK, N] payloads."""
    import ml_dtypes

    p32 = np.asarray(pc_b, dtype=np.float32)
    order = np.argsort(p32[:, AXIS], kind="stable")
    p = p32[order]
    xx = np.sum(p.astype(np.float64) ** 2, axis=1).astype(np.float32)
    ones = np.ones(N, np.float32)
    Lb = np.stack([2.0 * p[:, 0], 2.0 * p[:, 1], 2.0 * p[:, 2], xx, -ones])
    Rb = np.stack([p[:, 0], p[:, 1], p[:, 2], -ones, xx])
    bf16 = ml_dtypes.bfloat16
    Lh = Lb.astype(bf16)
    Ll = (Lb - Lh.astype(np.float32)).astype(bf16)
    Rh = Rb.astype(bf16)
    Rl = (Rb - Rh.astype(np.float32)).astype(bf16)
    # sum_k L[k]*R[k] = Lh.Rh + Lh.Rl + Ll.Rh (~fp32 product), one zero row
    zero = np.zeros((1, N), bf16)
    Lp = np.ascontiguousarray(np.concatenate([Lh, Lh, Ll, zero], axis=0))
    Rp = np.ascontiguousarray(np.concatenate([Rh, Rl, Rh, zero], axis=0))
    return {"order": order, "p": p.astype(np.float64), "L": Lp, "R": Rp}


def batch_values(val_tile, prep, w=W):
    """Per-point mean-5NN values (sorted order) + exact fixup of unproven rows."""
    # val[p, t*8+j] = j-th largest negdist of sorted point t*128+p
    top8 = (
        np.ascontiguousarray(val_tile)
        .reshape(P, NT, 8)
        .transpose(1, 0, 2)
        .reshape(N, 8)
        .astype(np.float64)
    )
    value = -np.mean(top8[:, 1 : 1 + KNN], axis=1)
    d5 = -top8[:, 1 + KNN - 1]  # squared distance to the 5th NN

    p = prep["p"]  # sorted, float64
    x = p[:, AXIS]
    lm = np.full(N, np.inf)
    rm = np.full(N, np.inf)
    for t in range(NT):
        lo = window_lo(t, w)
        hi = lo + w
        rows = slice(t * P, (t + 1) * P)
        if lo > 0:
            lm[rows] = x[rows] - x[lo]
        if hi < N:
            rm[rows] = x[hi - 1] - x[rows]
    m = np.minimum(lm, rm)
    bad = d5 >= (m * m) * (1.0 - 1e-3)
    if bad.any():
        idx = np.flatnonzero(bad)
        xx = np.sum(p * p, axis=1)
        db = xx[idx, None] + xx[None, :] - 2.0 * (p[idx] @ p.T)
        dsb = np.sort(db, axis=1)
        value[idx] = dsb[:, 1 : 1 + KNN].mean(axis=1)
    return value, int(bad.sum())


def finish_on_host(val_tiles, preps, weights, w=W):
    """Reference-exact epilogue: threshold stats + weighted mean, in f32."""
    losses = np.zeros(B, np.float32)
    wts = np.asarray(weights, dtype=np.float32)
    nfix = 0
    for b in range(B):
        v, nf = batch_values(val_tiles[b], preps[b], w)
        nfix += nf
        v = v.astype(np.float32)
        mean = np.mean(v, dtype=np.float32)
        var = np.sum((v - mean) ** 2, dtype=np.float32) / np.float32(N - 1)
        std = np.sqrt(var)
        thr = mean + ALPHA * std
        mask = (v > thr).astype(np.float32)
        losses[b] = np.mean(v * mask, dtype=np.float32) * wts[b]
    return np.array(np.mean(losses, dtype=np.float32), dtype=np.float32), nfix


def run_device(pc, w=W, **spmd_kwargs):
    nc = get_program(w)
    preps = [prep_batch(pc[b]) for b in range(B)]
    in_maps = [{"L": preps[b]["L"], "Rm": preps[b]["R"]} for b in range(B)]
    res = bass_utils.run_bass_kernel_spmd(
        nc, in_maps, core_ids=list(range(B)), **spmd_kwargs
    )
    vals = [res.results[b]["val"] for b in range(B)]
    return vals, preps, res


def kernel(pc, weights):
    pc = np.asarray(pc, dtype=np.float32)
    vals, preps, _ = run_device(pc)
    out, _ = finish_on_host(vals, preps, weights)
    return out
